# revision 10
# baseline (speedup 1.0000x reference)
"""HGDCODE (diffusion-conv GRU/ODE recurrence) Bass/Tile kernel for 8 trn2 cores.

Sharding: data-parallel over batch B=8 (one batch element per core).
Layout strategy per core (batch b):
  - States kept stacked S=[hv|zv] in transposed form ST [128(feat), 1024(node)]
    plus normal form S [128(node-part), 8(node-chunk), 128(feat)].
  - A and A2=A@A applied via "transposed-out" matmuls: stationary lhsT = state
    chunk (normal), moving rhs = A^T / A2^T rows -> psum (A@g)^T, (A2@g)^T.
  - Graph-conv weight combines run in transposed orientation with block-diag
    stacked [v|z] weights; biases ride the scalar-engine activation
    (per-partition bias = per-feature in transposed layout).
  - Normal-orientation copies (needed as next matmul stationaries) made with
    PE transposes (128x128 identity matmuls).
  - X diffusion (A@x_t, A2@x_t) hoisted: batched 8 timesteps per A-pass.
Outputs are written transposed [T, feat, node]; host transposes back.
"""

import os
import sys

for _p in ("/opt/trn_rl_repo",):
    if _p not in sys.path:
        sys.path.append(_p)

import numpy as np

import concourse.bass as bass
import concourse.mybir as mybir
import concourse.tile as tile
from concourse.bass_utils import run_bass_kernel_spmd
from concourse.masks import make_identity

F32 = mybir.dt.float32
AF = mybir.ActivationFunctionType
OP = mybir.AluOpType

B, T, N, DIN, DR = 8, 64, 1024, 16, 64
P = 128          # partitions
NC_ = 8          # node chunks (N // P)
D2 = 128         # stacked feature dim [hv|zv]
TG = 4           # timesteps per X-diffusion group
NG = T // TG     # number of X groups


def _split_multiwaits(nc):
    """Walrus in this env accepts at most ONE sync wait per instruction.
    Hoist extra waits onto injected same-engine NoOps placed just before."""
    cnt = 0
    for bb in nc.m.functions[0].blocks:
        il = bb.instructions
        i = 0
        while i < len(il):
            inst = il[i]
            si = inst.sync_info
            if si is not None and si.on_wait is not None and len(si.on_wait) > 1:
                waits = list(si.on_wait)
                for w in waits[:-1]:
                    cnt += 1
                    nop = mybir.InstNoOp(
                        name=f"I-ws{cnt}",
                        engine=inst.engine,
                        sync_info=mybir.SyncInfo(on_wait=[w], on_update=[]),
                    )
                    il.insert(i, nop)
                    i += 1
                inst.sync_info = mybir.SyncInfo(
                    on_wait=[waits[-1]], on_update=list(si.on_update or [])
                )
            i += 1
    return cnt


def build_nc(t_steps=T):
    nc = bass.Bass("TRN2")

    # ---- DRAM I/O ----
    AT = nc.dram_tensor("AT", [N, N], F32, kind="ExternalInput")
    A2T = nc.dram_tensor("A2T", [N, N], F32, kind="ExternalInput")
    XG = nc.dram_tensor("XG", [NG, NC_, P, TG * 32], F32, kind="ExternalInput")
    XT = nc.dram_tensor("XT", [T, DIN, N], F32, kind="ExternalInput")
    OBS = nc.dram_tensor("OBS", [T, 1, N], F32, kind="ExternalInput")
    S0N = nc.dram_tensor("S0N", [P, NC_, D2], F32, kind="ExternalInput")
    S0T = nc.dram_tensor("S0T", [P, N], F32, kind="ExternalInput")
    WODE = nc.dram_tensor("WODE", [3, 3, D2, D2], F32, kind="ExternalInput")
    BODE = nc.dram_tensor("BODE", [3, D2], F32, kind="ExternalInput")
    WGH = nc.dram_tensor("WGH", [3, 3, D2, D2], F32, kind="ExternalInput")
    WGX = nc.dram_tensor("WGX", [3, 96, D2], F32, kind="ExternalInput")
    BG = nc.dram_tensor("BG", [3, D2], F32, kind="ExternalInput")
    WMLP = nc.dram_tensor("WMLP", [3, DR, DR], F32, kind="ExternalInput")
    BMLP = nc.dram_tensor("BMLP", [3, DR], F32, kind="ExternalInput")
    WOUTP = nc.dram_tensor("WOUTP", [DR, DIN], F32, kind="ExternalInput")
    BOUTP = nc.dram_tensor("BOUTP", [DIN, 1], F32, kind="ExternalInput")

    HPRE = nc.dram_tensor("HPRE", [t_steps, DR, N], F32, kind="ExternalOutput")
    HPG = nc.dram_tensor("HPG", [t_steps, DR, N], F32, kind="ExternalOutput")
    HPOST = nc.dram_tensor("HPOST", [t_steps, DR, N], F32, kind="ExternalOutput")
    XPRED = nc.dram_tensor("XPRED", [t_steps, DIN, N], F32, kind="ExternalOutput")

    from contextlib import ExitStack
    with tile.TileContext(nc) as tc, ExitStack() as ctx:
        consts = ctx.enter_context(tc.tile_pool(name="consts", bufs=1))
        sb = ctx.enter_context(tc.tile_pool(name="sb", bufs=2))
        gn = ctx.enter_context(tc.tile_pool(name="gn", bufs=2))
        apg = ctx.enter_context(tc.tile_pool(name="apg", bufs=4, space="PSUM"))
        cmb = ctx.enter_context(tc.tile_pool(name="cmb", bufs=2, space="PSUM"))
        tpp = ctx.enter_context(tc.tile_pool(name="tpp", bufs=2, space="PSUM"))

        # ---- constants ----
        at_sb = consts.tile([P, NC_, N], F32, name="at_sb")
        nc.sync.dma_start(at_sb[:], AT.rearrange("(mc p) n -> p mc n", p=P))
        a2t_sb = consts.tile([P, NC_, N], F32, name="a2t_sb")
        nc.sync.dma_start(a2t_sb[:], A2T.rearrange("(mc p) n -> p mc n", p=P))

        wode_sb = consts.tile([D2, 9, D2], F32, name="wode_sb")
        nc.sync.dma_start(wode_sb[:], WODE.rearrange("s k d e -> d (s k) e"))
        bode_sb = consts.tile([D2, 3], F32, name="bode_sb")
        nc.sync.dma_start(bode_sb[:], BODE.rearrange("s d -> d s"))
        wgh_sb = consts.tile([D2, 9, D2], F32, name="wgh_sb")
        nc.sync.dma_start(wgh_sb[:], WGH.rearrange("g k d e -> d (g k) e"))
        wgx_sb = consts.tile([96, 3, D2], F32, name="wgx_sb")
        nc.sync.dma_start(wgx_sb[:], WGX.rearrange("g d e -> d g e"))
        bg_sb = consts.tile([D2, 3], F32, name="bg_sb")
        nc.sync.dma_start(bg_sb[:], BG.rearrange("g d -> d g"))
        wmlp_sb = consts.tile([DR, 3, DR], F32, name="wmlp_sb")
        nc.sync.dma_start(wmlp_sb[:], WMLP.rearrange("s d e -> d s e"))
        bmlp_sb = consts.tile([DR, 3], F32, name="bmlp_sb")
        nc.sync.dma_start(bmlp_sb[:], BMLP.rearrange("s d -> d s"))
        woutp_sb = consts.tile([DR, DIN], F32, name="woutp_sb")
        nc.sync.dma_start(woutp_sb[:], WOUTP[:])
        boutp_sb = consts.tile([DIN, 1], F32, name="boutp_sb")
        nc.sync.dma_start(boutp_sb[:], BOUTP[:])

        xstg = consts.tile([96, N], F32, name="xstg")
        nc.gpsimd.memset(xstg[:], 0.0)
        ident = consts.tile([P, P], F32, name="ident")
        make_identity(nc, ident[:])
        ones1 = consts.tile([1, P], F32, name="ones1")
        nc.gpsimd.memset(ones1[:], 1.0)

        s_nrm = gn.tile([P, NC_, D2], F32, tag="sn_n", name="s_nrm_init")
        nc.sync.dma_start(s_nrm[:], S0N[:])
        s_t = sb.tile([P, N], F32, tag="sn_t", bufs=2, name="s_t_init")
        nc.sync.dma_start(s_t[:], S0T[:])

        # ---- helpers ----
        def a_group(g_norm, tag, width=D2):
            """(A@g)^T, (A2@g)^T for g given in normal layout [P, NC_, width]."""
            p1 = apg.tile([P, N], F32, tag="apg", name="apg1")
            p2 = apg.tile([P, N], F32, tag="apg", name="apg2")
            for m in range(NC_):
                for ch in range(2):
                    nc.tensor.matmul(
                        p1[:width, ch * 512:(ch + 1) * 512], g_norm[:, m, :],
                        at_sb[:, m, ch * 512:(ch + 1) * 512],
                        start=(m == 0), stop=(m == NC_ - 1))
                    nc.tensor.matmul(
                        p2[:width, ch * 512:(ch + 1) * 512], g_norm[:, m, :],
                        a2t_sb[:, m, ch * 512:(ch + 1) * 512],
                        start=(m == 0), stop=(m == NC_ - 1))
            o1 = sb.tile([width, N], F32, tag=tag + "1", bufs=1, name=tag + "1")
            o2 = sb.tile([width, N], F32, tag=tag + "2", bufs=1, name=tag + "2")
            nc.scalar.copy(o1[:], p1[:width, :])
            nc.vector.tensor_copy(o2[:], p2[:width, :])
            return o1, o2

        def combine(terms, bias_ap, func, out_tag, e_width=D2):
            """out^T[e, n] = func(sum_i lhsT_i.T @ rhsT_i + bias). terms: (lhsT, rhsT)."""
            out = sb.tile([e_width, N], F32, tag=out_tag, bufs=1, name=out_tag)
            pt = cmb.tile([P, N], F32, tag="cmb", name="cmbp")
            for ch in range(2):
                for i, (lt, rt) in enumerate(terms):
                    nc.tensor.matmul(
                        pt[:e_width, ch * 512:(ch + 1) * 512], lt,
                        rt[:, ch * 512:(ch + 1) * 512],
                        start=(i == 0), stop=(i == len(terms) - 1))
            nc.scalar.activation(out[:], pt[:e_width, :],
                                 AF.Identity if func is None else func, bias=bias_ap)
            return out

        def to_normal(srcT, tag):
            """[D2, N] transposed -> [P, NC_, D2] normal, via PE transposes."""
            out = gn.tile([P, NC_, D2], F32, tag=tag, name=tag, bufs=(2 if tag=="sn_n" else 1))
            for ti in range(NC_):
                pt = tpp.tile([P, P], F32, tag="tpp", name="tppp")
                nc.tensor.transpose(pt[:], srcT[:, ti * P:(ti + 1) * P], ident[:])
                nc.vector.tensor_copy(out[:, ti, :], pt[:])
            return out

        # ---- time loop ----
        axt = a2xt = None
        for t in range(t_steps):
            if t % TG == 0:
                g = t // TG
                xg_sb = sb.tile([P, NC_, TG * 32], F32, tag="xg", bufs=1, name="xg_sb")
                nc.sync.dma_start(xg_sb[:], XG[g].rearrange("mc p f -> p mc f"))
                axt, a2xt = a_group(xg_sb, "axt", width=P)

            # x^T staging [96, N]: 32-aligned slots [x^T; (A x)^T; (A2 x)^T]
            nc.sync.dma_start(xstg[0:DIN, :], XT[t])
            toff = (t % TG) * 32
            nc.vector.tensor_copy(xstg[32:32 + DIN, :], axt[toff:toff + DIN, :])
            nc.vector.tensor_copy(xstg[64:64 + DIN, :], a2xt[toff:toff + DIN, :])

            obsr = sb.tile([1, N], F32, tag="obsr", bufs=2, name="obsr")
            nc.sync.dma_start(obsr[:], OBS[t])

            # ---- ODE: 3 chained gconv stages on S ----
            ast, a2st = a_group(s_nrm, "ag")
            y1t = combine(
                [(wode_sb[:, 0, :], s_t), (wode_sb[:, 1, :], ast), (wode_sb[:, 2, :], a2st)],
                bode_sb[:, 0:1], AF.Tanh, "y1t")
            y1n = to_normal(y1t, "gtmp")

            ay1, a2y1 = a_group(y1n, "ag")
            y2t = combine(
                [(wode_sb[:, 3, :], y1t), (wode_sb[:, 4, :], ay1), (wode_sb[:, 5, :], a2y1)],
                bode_sb[:, 1:2], AF.Tanh, "y2t")
            y2n = to_normal(y2t, "gtmp")

            ay2, a2y2 = a_group(y2n, "ag")
            ot = combine(
                [(wode_sb[:, 6, :], y2t), (wode_sb[:, 7, :], ay2), (wode_sb[:, 8, :], a2y2)],
                bode_sb[:, 2:3], None, "ot")

            # Euler step (DT=1): S1 = S + O
            s1t = sb.tile([P, N], F32, tag="s1t", bufs=2, name="s1t")
            nc.vector.tensor_tensor(s1t[:], s_t[:], ot[:], OP.add)
            s1n = to_normal(s1t, "gtmp")

            # hv_pre output
            nc.sync.dma_start(HPRE[t], s1t[0:DR, :])

            # ---- GRU ----
            as1, a2s1 = a_group(s1n, "ag")
            rt = combine(
                [(wgh_sb[:, 0, :], s1t), (wgh_sb[:, 1, :], as1), (wgh_sb[:, 2, :], a2s1),
                 (wgx_sb[:, 0, :], xstg)],
                bg_sb[:, 0:1], AF.Sigmoid, "rt")
            ut = combine(
                [(wgh_sb[:, 3, :], s1t), (wgh_sb[:, 4, :], as1), (wgh_sb[:, 5, :], a2s1),
                 (wgx_sb[:, 1, :], xstg)],
                bg_sb[:, 1:2], AF.Sigmoid, "ut")

            rct = sb.tile([P, N], F32, tag="rct", bufs=1, name="rct")
            nc.vector.tensor_tensor(rct[:], rt[:], s1t[:], OP.mult)
            rcn = to_normal(rct, "gtmp")

            arc, a2rc = a_group(rcn, "ag")
            ct = combine(
                [(wgh_sb[:, 6, :], rct), (wgh_sb[:, 7, :], arc), (wgh_sb[:, 8, :], a2rc),
                 (wgx_sb[:, 2, :], xstg)],
                bg_sb[:, 2:3], AF.Tanh, "ct")

            # blended update: Sn = S1 + obs*(u-1)*(S1 - c)
            d1 = sb.tile([P, N], F32, tag="d1", bufs=1, name="d1")
            nc.vector.tensor_tensor(d1[:], s1t[:], ct[:], OP.subtract)
            u1 = sb.tile([P, N], F32, tag="u1", bufs=1, name="u1")
            nc.vector.tensor_scalar_add(u1[:], ut[:], -1.0)
            m1 = sb.tile([P, N], F32, tag="m1", bufs=1, name="m1")
            nc.vector.tensor_tensor(m1[:], u1[:], d1[:], OP.mult)
            # broadcast obs over partitions via ones-matmul
            obs_ps = cmb.tile([P, N], F32, tag="cmb", name="obsps")
            for ch in range(2):
                nc.tensor.matmul(obs_ps[:, ch * 512:(ch + 1) * 512], ones1[:],
                                 obsr[:, ch * 512:(ch + 1) * 512],
                                 start=True, stop=True)
            sn_t = sb.tile([P, N], F32, tag="sn_t", bufs=2, name="sn_t")
            nc.vector.tensor_tensor(m1[:], obs_ps[:], m1[:], OP.mult)
            nc.vector.tensor_tensor(sn_t[:], s1t[:], m1[:], OP.add)
            sn_n = to_normal(sn_t, "sn_n")

            # ---- gate MLP + outputs ----
            zvt = sb.tile([DR, N], F32, tag="zvt", bufs=1, name="zvt")
            nc.vector.tensor_copy(zvt[:], s1t[DR:D2, :])
            g1t = combine([(wmlp_sb[:, 0, :], zvt)],
                          bmlp_sb[:, 0:1], AF.Tanh, "g1t", e_width=DR)
            g2t = combine([(wmlp_sb[:, 1, :], g1t)],
                          bmlp_sb[:, 1:2], AF.Tanh, "g2t", e_width=DR)
            gat = combine([(wmlp_sb[:, 2, :], g2t)],
                          bmlp_sb[:, 2:3], AF.Sigmoid, "gat", e_width=DR)

            hpg = sb.tile([DR, N], F32, tag="hpg", bufs=1, name="hpg")
            nc.vector.tensor_tensor(hpg[:], s1t[0:DR, :], gat[:], OP.mult)
            hpo = sb.tile([DR, N], F32, tag="hpo", bufs=1, name="hpo")
            nc.vector.tensor_tensor(hpo[:], sn_t[0:DR, :], gat[:], OP.mult)
            xpr = combine([(woutp_sb[:], hpg)], boutp_sb[:, 0:1], None,
                          "xpr", e_width=DIN)

            nc.sync.dma_start(HPG[t], hpg[:])
            nc.sync.dma_start(HPOST[t], hpo[:])
            nc.sync.dma_start(XPRED[t], xpr[:])

            s_t = sn_t
            s_nrm = sn_n

    _split_multiwaits(nc)
    return nc


def _prep_host(values, masks, A, t, params, t_steps=T):
    """Build per-core input maps (numpy)."""
    A = np.asarray(A, np.float32)
    X = (np.asarray(values, np.float32) * np.asarray(masks, np.float32))
    obs = (np.sum(np.abs(np.asarray(masks, np.float32)), axis=-1) > 1e-4)
    obs = obs.astype(np.float32)  # [B, T, N]

    A2 = (A @ A).astype(np.float32)
    AT = np.ascontiguousarray(A.T)
    A2T = np.ascontiguousarray(A2.T)

    p = {k: {kk: np.asarray(vv, np.float32) if not isinstance(vv, list)
             else [np.asarray(x, np.float32) for x in vv]
             for kk, vv in v.items()} if isinstance(v, dict) else np.asarray(v, np.float32)
         for k, v in params.items()}

    def blkdiag(a, b):
        o = np.zeros((a.shape[0] + b.shape[0], a.shape[1] + b.shape[1]), np.float32)
        o[:a.shape[0], :a.shape[1]] = a
        o[a.shape[0]:, a.shape[1]:] = b
        return o

    ov, oz = p["ode_v"], p["ode_g"]
    ode_stages = [
        (ov["W_in"], oz["W_in"], ov["b_in"], oz["b_in"]),
        (ov["W_h"][0], oz["W_h"][0], ov["b_h"][0], oz["b_h"][0]),
        (ov["W_out"], oz["W_out"], ov["b_out"], oz["b_out"]),
    ]
    WODE = np.stack([np.stack([blkdiag(wv[k], wz[k]) for k in range(3)])
                     for (wv, wz, _, _) in ode_stages])  # [3,3,128,128]
    BODE = np.stack([np.concatenate([bv, bz]) for (_, _, bv, bz) in ode_stages])

    gv, gz = p["gru_v"], p["gru_g"]
    gates = [("Wr", "br"), ("Wu", "bu"), ("Wc", "bc")]
    WGH = np.stack([np.stack([blkdiag(gv[w][k][DIN:], gz[w][k][DIN:])
                              for k in range(3)]) for (w, _) in gates])
    WGX = np.zeros((3, 96, D2), np.float32)
    for gi, (w, _) in enumerate(gates):
        for k in range(3):
            WGX[gi, k * 32:k * 32 + DIN] = np.concatenate(
                [gv[w][k][:DIN], gz[w][k][:DIN]], axis=1)
    BG = np.stack([np.concatenate([gv[b], gz[b]]) for (_, b) in gates])

    z = p["zout"]
    WMLP = np.stack([z["W1"], z["W2"], z["W3"]])
    BMLP = np.stack([z["b1"], z["b2"], z["b3"]])
    WOUTP = p["out"]["W"]
    BOUTP = p["out"]["b"].reshape(DIN, 1)

    h0 = p["h0"].reshape(DR)
    z0 = p["z0"].reshape(DR)
    s0row = np.concatenate([h0, z0])  # [128]
    S0N = np.broadcast_to(s0row, (P, NC_, D2)).astype(np.float32).copy()
    S0T = np.repeat(s0row.reshape(P, 1), N, axis=1).astype(np.float32).copy()

    shared = dict(AT=AT, A2T=A2T, WODE=WODE, BODE=BODE, WGH=WGH, WGX=WGX,
                  BG=BG, WMLP=WMLP, BMLP=BMLP, WOUTP=WOUTP, BOUTP=BOUTP,
                  S0N=S0N, S0T=S0T)
    shared = {k: np.ascontiguousarray(v, np.float32) for k, v in shared.items()}

    in_maps = []
    for b in range(B):
        Xb = X[b][:t_steps]  # [t, N, DIN]
        ng_b = t_steps // TG
        Zb = np.zeros((ng_b, TG, N, 32), np.float32)
        Zb[..., :DIN] = Xb.reshape(ng_b, TG, N, DIN)
        XGb = np.transpose(Zb, (0, 2, 1, 3)).reshape(-1, NC_, P, TG * 32)
        if XGb.shape[0] < NG:  # pad groups for smaller debug t_steps
            pad = np.zeros((NG - XGb.shape[0], NC_, P, TG * 32), np.float32)
            XGb = np.concatenate([XGb, pad])
        XTb = np.transpose(Xb, (0, 2, 1))  # [t, DIN, N]
        if XTb.shape[0] < T:
            XTb = np.concatenate([XTb, np.zeros((T - XTb.shape[0], DIN, N), np.float32)])
        OBSb = obs[b][:, None, :]  # [T, 1, N]
        in_maps.append(dict(shared,
                            XG=np.ascontiguousarray(XGb, np.float32),
                            XT=np.ascontiguousarray(XTb, np.float32),
                            OBS=np.ascontiguousarray(OBSb, np.float32)))
    return in_maps


_NC_CACHE = {}


def kernel(values, masks, A, t, params, _trace=False, _t_steps=None):
    t_steps = _t_steps or int(os.environ.get("BASS_T", T))
    if t_steps not in _NC_CACHE:
        _NC_CACHE[t_steps] = build_nc(t_steps)
    nc = _NC_CACHE[t_steps]

    in_maps = _prep_host(values, masks, A, t, params, t_steps)
    res = run_bass_kernel_spmd(nc, in_maps, core_ids=list(range(B)), trace=_trace)

    hv_pre = np.stack([np.transpose(res.results[b]["HPRE"], (0, 2, 1)) for b in range(B)])
    hv_pre_g = np.stack([np.transpose(res.results[b]["HPG"], (0, 2, 1)) for b in range(B)])
    hv_post_g = np.stack([np.transpose(res.results[b]["HPOST"], (0, 2, 1)) for b in range(B)])
    x_pred = np.stack([np.transpose(res.results[b]["XPRED"], (0, 2, 1)) for b in range(B)])

    out = (x_pred[:, 1:], hv_pre, hv_pre_g, hv_post_g,
           np.asarray(t, np.float32))
    if _trace:
        return out, res
    return out
